# revision 8
# baseline (speedup 1.0000x reference)
"""AttributeMemoryFusion kernel for 8x TRN2 NeuronCores (Bass/Tile).

Per-sample attention over ragged memory + gated fusion:
    scores = mem @ h ; attn = softmax(mask(scores)) ; r = attn @ mem
    g = sigmoid(h @ Wg.T + r @ Ug.T + b) ; out = where(len>0, g*r+(1-g)*h, h)

Data-parallel over batch: B=8192 split as 1024 samples/core across 8 cores.

Layout: batch in SBUF partitions (128-sample tiles). scores / r are per-
partition contractions along the free dim, done as chains of fused
scalar_tensor_tensor ops on VectorE (accum_out gives the dot product; the
r chain accumulates in-place). Gate matmuls run on TensorE in transposed
[d, b] layout (PE transposes for h/r, transpose back for g); the
empty-memory passthrough is folded into the gate preactivation with a
rank-1 matmul that adds -1e9 to empty samples' logits (sigmoid -> 0).
"""

from contextlib import ExitStack

import numpy as np

import concourse.bass as bass
import concourse.bacc as bacc
import concourse.mybir as mybir
import concourse.tile as tile
from concourse import masks
from concourse.bass_utils import run_bass_kernel_spmd

B, M, D = 8192, 64, 256
N_CORES = 8
BC = B // N_CORES      # samples per core
P = 128                # partitions / samples per tile
N_TILES = BC // P
BIG = 1.0e9

F32 = mybir.dt.float32
I32 = mybir.dt.int32
Alu = mybir.AluOpType
Act = mybir.ActivationFunctionType
AX = mybir.AxisListType


def _build_body(ctx, tc, io):
    nc = tc.nc
    h_ap, mem_ap, len_ap, wg_ap, wgb_ap, ug_ap, ugb_ap, bg_ap, out_ap = io

    # ---- one-time constants ----
    const = ctx.enter_context(tc.tile_pool(name="const", bufs=1))
    ident = const.tile([P, P], F32)
    masks.make_identity(nc, ident[:])
    iota_m = const.tile([P, M], F32)
    nc.gpsimd.iota(
        iota_m[:], pattern=[[1, M]], base=0, channel_multiplier=0,
        allow_small_or_imprecise_dtypes=True,
    )
    ones_row = const.tile([1, P], F32)
    nc.vector.memset(ones_row[:], 1.0)

    # ---- weights: load natural [o,i], transpose to lhsT layout [i_in, i_blk, o] ----
    wpool = ctx.enter_context(tc.tile_pool(name="weights", bufs=1))
    wg_nat = wpool.tile([P, 2, D], F32)
    ug_nat = wpool.tile([P, 2, D], F32)
    nc.sync.dma_start(wg_nat[:], wg_ap.rearrange("(a p) i -> p a i", p=P))
    nc.sync.dma_start(ug_nat[:], ug_ap.rearrange("(a p) i -> p a i", p=P))
    wgT = wpool.tile([P, 2, D], F32)
    ugT = wpool.tile([P, 2, D], F32)
    with tc.tile_pool(name="psw", bufs=2, space="PSUM") as psw:
        for nat, T in ((wg_nat, wgT), (ug_nat, ugT)):
            for ob in range(2):
                for ib in range(2):
                    pt = psw.tile([P, P], F32, tag="wtr")
                    nc.tensor.transpose(pt[:], nat[:, ob, ib * P:(ib + 1) * P], ident[:])
                    nc.scalar.copy(T[:, ib, ob * P:(ob + 1) * P], pt[:])

    # summed gate bias in transposed layout: [o_in, o_blk]
    bt0 = wpool.tile([P, 2], F32)
    bt1 = wpool.tile([P, 2], F32)
    bt2 = wpool.tile([P, 2], F32)
    bias_sb = wpool.tile([P, 2], F32)
    nc.sync.dma_start(bt0[:], wgb_ap.rearrange("(a p) -> p a", p=P))
    nc.sync.dma_start(bt1[:], ugb_ap.rearrange("(a p) -> p a", p=P))
    nc.sync.dma_start(bt2[:], bg_ap.rearrange("(a p) -> p a", p=P))
    nc.vector.tensor_add(bias_sb[:], bt0[:], bt1[:])
    nc.vector.tensor_add(bias_sb[:], bias_sb[:], bt2[:])

    # ---- pools ----
    mem_pool = ctx.enter_context(tc.tile_pool(name="mem", bufs=2))
    h_pool = ctx.enter_context(tc.tile_pool(name="h", bufs=2))
    len_pool = ctx.enter_context(tc.tile_pool(name="len", bufs=2))
    small = ctx.enter_context(tc.tile_pool(name="small", bufs=2))
    out_pool = ctx.enter_context(tc.tile_pool(name="out", bufs=2))
    ps = ctx.enter_context(tc.tile_pool(name="ps", bufs=2, space="PSUM"))

    for t in range(N_TILES):
        b0 = t * P
        # ---- loads (mem split into 8 DMAs to spread queues) ----
        mt = mem_pool.tile([P, M, D], F32)
        for c in range(8):
            nc.sync.dma_start(
                mt[:, c * 8:(c + 1) * 8, :], mem_ap[b0:b0 + P, c * 8:(c + 1) * 8, :]
            )
        ht = h_pool.tile([P, D], F32)
        nc.sync.dma_start(ht[:], h_ap[b0:b0 + P, :])
        lt = len_pool.tile([P, 1], I32)
        nc.sync.dma_start(lt[:], len_ap[b0:b0 + P].rearrange("(p one) -> p one", one=1))
        lrow = len_pool.tile([1, P], I32)
        nc.sync.dma_start(lrow[:], len_ap[b0:b0 + P].rearrange("(one p) -> one p", one=1))

        # ---- scores[b, m] = <mem[b, m, :], h[b, :]> ----
        scratch = small.tile([P, D], F32, tag="scratch")
        S = small.tile([P, M], F32, tag="S")
        for m in range(M):
            nc.vector.scalar_tensor_tensor(
                out=scratch[:], in0=mt[:, m, :], scalar=1.0, in1=ht[:],
                op0=Alu.mult, op1=Alu.mult, accum_out=S[:, m:m + 1],
            )

        # ---- masked softmax over m ----
        ltf = small.tile([P, 1], F32, tag="ltf")
        nc.vector.tensor_copy(ltf[:], lt[:])
        maskf = small.tile([P, M], F32, tag="maskf")
        nc.vector.tensor_scalar(maskf[:], iota_m[:], ltf[:], None, Alu.is_lt)
        negm = small.tile([P, M], F32, tag="negm")
        nc.vector.tensor_scalar(negm[:], maskf[:], BIG, BIG, Alu.mult, Alu.subtract)
        Sm = small.tile([P, M], F32, tag="Sm")
        nc.vector.tensor_tensor(Sm[:], S[:], maskf[:], Alu.mult)
        nc.vector.tensor_tensor(Sm[:], Sm[:], negm[:], Alu.add)
        negmax = small.tile([P, 1], F32, tag="negmax")
        nc.vector.tensor_reduce(negmax[:], Sm[:], AX.X, Alu.max, negate=True)
        E = small.tile([P, M], F32, tag="E")
        nc.scalar.activation(E[:], Sm[:], Act.Exp, bias=negmax[:], scale=1.0)
        ssum = small.tile([P, 1], F32, tag="ssum")
        nc.vector.tensor_reduce(ssum[:], E[:], AX.X, Alu.add)
        rinv = small.tile([P, 1], F32, tag="rinv")
        nc.vector.reciprocal(rinv[:], ssum[:])
        attn = small.tile([P, M], F32, tag="attn")
        nc.vector.tensor_scalar(attn[:], E[:], rinv[:], None, Alu.mult)

        # ---- r[b, :] = sum_m attn[b, m] * mem[b, m, :] ----
        R = small.tile([P, D], F32, tag="R")
        nc.vector.memset(R[:], 0.0)
        for m in range(M):
            nc.vector.scalar_tensor_tensor(
                out=R[:], in0=mt[:, m, :], scalar=attn[:, m:m + 1], in1=R[:],
                op0=Alu.mult, op1=Alu.add,
            )

        # ---- transpose h, r to [d, b] for the gate matmuls ----
        pt_h = ps.tile([P, 2, P], F32, tag="pth")
        pt_r = ps.tile([P, 2, P], F32, tag="ptr")
        hT = small.tile([P, 2, P], F32, tag="hT")
        rT = small.tile([P, 2, P], F32, tag="rT")
        for k in range(2):
            nc.tensor.transpose(pt_h[:, k, :], ht[:, k * P:(k + 1) * P], ident[:])
            nc.scalar.copy(hT[:, k, :], pt_h[:, k, :])
            nc.tensor.transpose(pt_r[:, k, :], R[:, k * P:(k + 1) * P], ident[:])
            nc.scalar.copy(rT[:, k, :], pt_r[:, k, :])

        # empty-sample logit penalty, as a row vector over b
        lrowf = small.tile([1, P], F32, tag="lrowf")
        nc.vector.tensor_copy(lrowf[:], lrow[:])
        hmrow = small.tile([1, P], F32, tag="hmrow")
        nc.vector.tensor_scalar(hmrow[:], lrowf[:], 0.0, None, Alu.is_gt)
        negrow = small.tile([1, P], F32, tag="negrow")
        nc.vector.tensor_scalar(negrow[:], hmrow[:], BIG, BIG, Alu.mult, Alu.subtract)

        # ---- gate preactivation in PSUM: Wg@hT + Ug@rT + ones x negrow ----
        G = ps.tile([P, 2, P], F32, tag="G")
        for ob in range(2):
            for ib in range(2):
                nc.tensor.matmul(
                    G[:, ob, :], wgT[:, ib, ob * P:(ob + 1) * P], hT[:, ib, :],
                    start=(ib == 0), stop=False,
                )
            for ib in range(2):
                nc.tensor.matmul(
                    G[:, ob, :], ugT[:, ib, ob * P:(ob + 1) * P], rT[:, ib, :],
                    start=False, stop=False,
                )
            nc.tensor.matmul(G[:, ob, :], ones_row[:], negrow[:], start=False, stop=True)

        gT = small.tile([P, 2, P], F32, tag="gT")
        for ob in range(2):
            nc.scalar.activation(
                gT[:, ob, :], G[:, ob, :], Act.Sigmoid,
                bias=bias_sb[:, ob:ob + 1], scale=1.0,
            )

        # transpose g back to [b, d]
        GB = ps.tile([P, 2, P], F32, tag="GB")
        for ob in range(2):
            nc.tensor.transpose(GB[:, ob, :], gT[:, ob, :], ident[:])

        # ---- out = h + g * (r - h) ----
        T1 = small.tile([P, D], F32, tag="T1")
        nc.vector.tensor_tensor(T1[:], R[:], ht[:], Alu.subtract)
        T2 = small.tile([P, D], F32, tag="T2")
        nc.vector.tensor_tensor(T2[:], T1[:], GB[:].rearrange("p a b -> p (a b)"), Alu.mult)
        ot = out_pool.tile([P, D], F32)
        nc.vector.tensor_tensor(ot[:], T2[:], ht[:], Alu.add)
        nc.sync.dma_start(out_ap[b0:b0 + P, :], ot[:])


_CACHE = {}


def _get_nc():
    if "nc" in _CACHE:
        return _CACHE["nc"]
    nc = bacc.Bacc("TRN2", target_bir_lowering=False, debug=False, num_devices=N_CORES)
    h_ap = nc.dram_tensor("h_tilde", [BC, D], F32, kind="ExternalInput").ap()
    mem_ap = nc.dram_tensor("mem", [BC, M, D], F32, kind="ExternalInput").ap()
    len_ap = nc.dram_tensor("lengths", [BC], I32, kind="ExternalInput").ap()
    wg_ap = nc.dram_tensor("Wg_w", [D, D], F32, kind="ExternalInput").ap()
    wgb_ap = nc.dram_tensor("Wg_b", [D], F32, kind="ExternalInput").ap()
    ug_ap = nc.dram_tensor("Ug_w", [D, D], F32, kind="ExternalInput").ap()
    ugb_ap = nc.dram_tensor("Ug_b", [D], F32, kind="ExternalInput").ap()
    bg_ap = nc.dram_tensor("b_g", [D], F32, kind="ExternalInput").ap()
    out_ap = nc.dram_tensor("out", [BC, D], F32, kind="ExternalOutput").ap()
    io = (h_ap, mem_ap, len_ap, wg_ap, wgb_ap, ug_ap, ugb_ap, bg_ap, out_ap)
    with tile.TileContext(nc) as tc:
        with ExitStack() as ctx:
            _build_body(ctx, tc, io)
    nc.finalize()
    _CACHE["nc"] = nc
    return nc


def _make_in_maps(inputs):
    h = np.ascontiguousarray(np.asarray(inputs["h_tilde"], dtype=np.float32))
    mem = np.ascontiguousarray(np.asarray(inputs["mem"], dtype=np.float32))
    lengths = np.ascontiguousarray(np.asarray(inputs["lengths"], dtype=np.int32))
    shared = {
        "Wg_w": np.ascontiguousarray(np.asarray(inputs["Wg_w"], dtype=np.float32)),
        "Wg_b": np.ascontiguousarray(np.asarray(inputs["Wg_b"], dtype=np.float32)),
        "Ug_w": np.ascontiguousarray(np.asarray(inputs["Ug_w"], dtype=np.float32)),
        "Ug_b": np.ascontiguousarray(np.asarray(inputs["Ug_b"], dtype=np.float32)),
        "b_g": np.ascontiguousarray(np.asarray(inputs["b_g"], dtype=np.float32)),
    }
    in_maps = []
    for c in range(N_CORES):
        sl = slice(c * BC, (c + 1) * BC)
        in_maps.append({
            "h_tilde": h[sl],
            "mem": mem[sl],
            "lengths": lengths[sl],
            **shared,
        })
    return in_maps


def run(inputs, **kwargs):
    nc = _get_nc()
    in_maps = _make_in_maps(inputs)
    return run_bass_kernel_spmd(nc, in_maps, list(range(N_CORES)), **kwargs)


def kernel(**inputs) -> np.ndarray:
    res = run(inputs)
    return np.concatenate([res.results[c]["out"] for c in range(N_CORES)], axis=0)
